# revision 14
# baseline (speedup 1.0000x reference)
"""CapsuleLayer (dynamic routing) Trainium2 kernel.

Problem: x [64,1152,8] f32, W [1152,64,8,16] f32 ->
  u_hat = einsum('bid,iodc->bioc', x, W)
  3 routing iterations (softmax over o=64, weighted i-sum, squash, agreement)
  returns v [64,64,16] f32.

Sharding: data-parallel over batch, 8 batch elements per core x 8 cores.

Per-core device strategy (all static, raw bass, manual semaphores):
  Phase 1 (PE + DMA + ACT):
    - Stream 72 fused tiles WX[g] = [W_tile | xbd_tile] fp16 [128,1152].
      W_tile rows = (i_sub*8+d), cols = (c*64+o); xbd = block-diag x,
      cols = (i_sub*8+b).
    - gen: psum[128=(i_sub,b), 1024=(c,o)] = xbd.T @ W_tile  (u_hat for
      16 in-caps x 8 batch) -> ACT copies psum -> u_hat fp16 SBUF
      [128, 72*1024] (free order g,c,o).
    - s0: psum_s[8,1024] += (xdn/64).T @ W_tile accumulated over g
      (c0 uniform = 1/64 folded into xdn).
  Routing iters t=0,1 over 18 chunks of 4 groups:
    - DVE: tmpa = u_chunk * vrep (fp16 2x); tree-reduce over c (16->1,
      in-place strided adds); logits L += agr (f32).
    - ACT: e = Exp(L_chunk) f32 (max-subtract not needed: |L| < 3).
    - DVE: Z = sum_o e; zr = 1/Z; c = e*zr fp16; tmps = u_chunk * c_bcast.
    - PE: selector matmuls (reduce i_sub partitions) accumulate
      s[8, 1024=(c,o)] in psum over all chunks.
  Squash after each s (ACT square/sqrt + DVE smalls), v replicated to
  128 partitions via 16 small SBUF->SBUF DMAs.
"""

import numpy as np

NB = 8        # batch per core
NCORES = 8
G = 72        # i-groups of 16 in-capsules
CG = 4        # groups per routing chunk
CH = G // CG  # 18 chunks
O, C, D = 64, 16, 8
ISUB = 16     # in-caps per group
F16 = None    # set after mybir import

_cache = {}


def _build_program():
    import concourse.bass as bass
    import concourse.mybir as mybir

    f16 = mybir.dt.float16
    f32 = mybir.dt.float32

    nc = bass.Bass('TRN2', target_bir_lowering=False, debug=False)

    # ---- DRAM I/O ----
    WX = nc.dram_tensor('WX', [G, 128, 1152], f16, kind='ExternalInput')
    XDN = nc.dram_tensor('XDN', [128, G * NB], f16, kind='ExternalInput')
    SEL = nc.dram_tensor('SEL', [128, NB], f16, kind='ExternalInput')
    VOUT = nc.dram_tensor('VOUT', [NB, 1024], f32, kind='ExternalOutput')

    # ---- SBUF ----
    u = nc.alloc_sbuf_tensor('u', [128, G * 1024], f16)          # 144KB/part
    wx0 = nc.alloc_sbuf_tensor('wx0', [128, 1152], f16)
    wx1 = nc.alloc_sbuf_tensor('wx1', [128, 1152], f16)
    xdn = nc.alloc_sbuf_tensor('xdn', [128, G * NB], f16)
    sel = nc.alloc_sbuf_tensor('sel', [128, NB], f16)
    L = nc.alloc_sbuf_tensor('L', [128, G * O], f32)             # 18KB
    Ltmp = nc.alloc_sbuf_tensor('Ltmp', [128, CG * O], f32)
    cbuf = nc.alloc_sbuf_tensor('cbuf', [128, CG * O], f16)
    tmpa = nc.alloc_sbuf_tensor('tmpa', [128, CG * 1024], f16)   # 8KB
    tmps0 = nc.alloc_sbuf_tensor('tmps0', [128, CG * 1024], f16)
    tmps1 = nc.alloc_sbuf_tensor('tmps1', [128, CG * 1024], f16)
    eb0 = nc.alloc_sbuf_tensor('eb0', [128, CG * O], f32)
    eb1 = nc.alloc_sbuf_tensor('eb1', [128, CG * O], f32)
    Zb = nc.alloc_sbuf_tensor('Zb', [128, G], f32)
    zr = nc.alloc_sbuf_tensor('zr', [128, G], f32)
    vrep = nc.alloc_sbuf_tensor('vrep', [128, 1024], f16)
    v16 = nc.alloc_sbuf_tensor('v16', [NB, 1024], f16)
    s2 = nc.alloc_sbuf_tensor('s2', [NB, 1024], f32)             # also vf
    sq = nc.alloc_sbuf_tensor('sq', [NB, O], f32)
    rr = nc.alloc_sbuf_tensor('rr', [NB, O], f32)
    q1 = nc.alloc_sbuf_tensor('q1', [NB, O], f32)
    q2 = nc.alloc_sbuf_tensor('q2', [NB, O], f32)
    q3 = nc.alloc_sbuf_tensor('q3', [NB, O], f32)
    qr = nc.alloc_sbuf_tensor('qr', [NB, O], f32)
    ff = nc.alloc_sbuf_tensor('ff', [NB, O], f32)
    vf = s2  # s2's last read (the sq reduce) precedes the vf write

    # ---- PSUM ----
    pg0 = nc.alloc_psum_tensor('pg0', [128, 1024], f32)
    pg1 = nc.alloc_psum_tensor('pg1', [128, 1024], f32)
    ps = nc.alloc_psum_tensor('ps', [NB, 1024], f32)

    pg = [pg0, pg1]
    wx = [wx0, wx1]
    tmps = [tmps0, tmps1]
    eb = [eb0, eb1]

    AF = mybir.ActivationFunctionType
    ALU = mybir.AluOpType
    AX = mybir.AxisListType

    sems = {}
    for name in ['d0', 'wxsemA', 'wxsemB', 'wxfree', 'pgsem', 'evsem',
                 'ssem', 'qa', 'qb', 'qc', 'sqdone', 'v16sem', 'vfsem',
                 'vrsem', 'Lsem', 'xsem', 'ebfree', 'smsem', 'tmpsfree',
                 'dout']:
        sems[name] = nc.alloc_semaphore(name)
    S = type('S', (), sems)

    def ap3(t, base, dims):
        # strided view: dims = [(step, count), ...] on free axis of [128,*]
        a = t.ap()
        return bass.AP(a.tensor, base, [a.ap[0]] + [[s, n] for s, n in dims])

    with nc.allow_low_precision(reason='fp16 tree adds validated to 5e-4'), \
         nc.Block() as block:

        # ---------------- SYNC: all DMA ----------------
        @block.sync
        def _(eng):
            eng.dma_start(sel.ap(), SEL.ap()).then_inc(S.d0, 16)
            eng.dma_start(xdn.ap(), XDN.ap()).then_inc(S.d0, 16)
            for g in range(G):
                if g >= 2:
                    eng.wait_ge(S.wxfree, g - 1)
                # parity sems: at most one outstanding DMA per sem, so the
                # cumulative wait value identifies the specific tile.
                wsem = S.wxsemA if g % 2 == 0 else S.wxsemB
                eng.dma_start(wx[g % 2].ap(), WX.ap()[g]).then_inc(wsem, 16)
            for t in range(2):
                eng.wait_ge(S.v16sem, t + 1)
                if t == 1:
                    eng.wait_ge(S.smsem, CH)  # iter-0 reads of vrep done
                for isub in range(ISUB):
                    eng.dma_start(vrep.ap()[isub * NB:(isub + 1) * NB, :],
                                  v16.ap()).then_inc(S.vrsem, 16)
            eng.wait_ge(S.vfsem, 1)
            eng.dma_start(VOUT.ap(), vf.ap()).then_inc(S.dout, 16)

        # ---------------- PE ----------------
        @block.tensor
        def _(eng):
            for g in range(G):
                b = g % 2
                wsem = S.wxsemA if b == 0 else S.wxsemB
                eng.wait_ge(wsem, 16 * (g // 2 + 1))
                if g >= 2:
                    eng.wait_ge(S.evsem, g - 1)
                eng.matmul(pg[b].ap()[:, 0:512],
                           lhsT=wx[b].ap()[:, 1024:1152],
                           rhs=wx[b].ap()[:, 0:512], start=True, stop=True)
                eng.matmul(pg[b].ap()[:, 512:1024],
                           lhsT=wx[b].ap()[:, 1024:1152],
                           rhs=wx[b].ap()[:, 512:1024],
                           start=True, stop=True).then_inc(S.pgsem, 1)
                if g == 0:
                    eng.wait_ge(S.d0, 32)
                eng.matmul(ps.ap()[:, 0:512],
                           lhsT=xdn.ap()[:, g * NB:(g + 1) * NB],
                           rhs=wx[b].ap()[:, 0:512],
                           start=(g == 0), stop=(g == G - 1))
                mm = eng.matmul(ps.ap()[:, 512:1024],
                                lhsT=xdn.ap()[:, g * NB:(g + 1) * NB],
                                rhs=wx[b].ap()[:, 512:1024],
                                start=(g == 0), stop=(g == G - 1))
                mm.then_inc(S.wxfree, 1)
                if g == G - 1:
                    eng.maybe_drain_then_inc((S.ssem, 1))
            for t in range(2):
                eng.wait_ge(S.sqdone, t + 1)
                for k in range(CH):
                    n = t * CH + k
                    eng.wait_ge(S.smsem, n + 1)
                    for gs in range(CG):
                        for h in range(2):
                            mm = eng.matmul(
                                ps.ap()[:, h * 512:(h + 1) * 512],
                                lhsT=sel.ap(),
                                rhs=tmps[k % 2].ap()[:, gs * 1024 + h * 512:
                                                     gs * 1024 + (h + 1) * 512],
                                start=(k == 0 and gs == 0),
                                stop=(k == CH - 1 and gs == CG - 1))
                    mm.then_inc(S.tmpsfree, 1)
                    if k == CH - 1:
                        eng.maybe_drain_then_inc((S.ssem, 1))

        # ---------------- ACT (scalar) ----------------
        @block.scalar
        def _(eng):
            def squash_act(t):
                eng.wait_ge(S.ssem, t + 1)
                eng.activation(s2.ap(), ps.ap(), AF.Square).then_inc(S.qa, 1)
                eng.wait_ge(S.qb, t + 1)
                eng.activation(rr.ap(), sq.ap(), AF.Sqrt).then_inc(S.qc, 1)

            for g in range(G):
                eng.wait_ge(S.pgsem, g + 1)
                eng.activation(
                    ap3(u, g * 1024, [(1, 1024)]),
                    pg[g % 2].ap(), AF.Copy).then_inc(S.evsem, 1)
            squash_act(0)
            for t in range(2):
                for k in range(CH):
                    n = t * CH + k
                    eng.wait_ge(S.Lsem, n + 1)
                    if n >= 2:
                        eng.wait_ge(S.ebfree, n - 1)
                    eng.activation(eb[n % 2].ap(),
                                   ap3(L, k * CG * O, [(1, CG * O)]),
                                   AF.Exp).then_inc(S.xsem, 1)
                squash_act(t + 1)

        # ---------------- DVE (vector) ----------------
        @block.vector
        def _(eng):
            def squash_dve(t):
                eng.wait_ge(S.qa, t + 1)
                # sq[b,o] = sum_c s2[b, c*64+o]
                eng.reduce_sum(sq.ap(),
                               ap3(s2, 0, [(1, O), (O, C)]),
                               axis=AX.X).then_inc(S.qb, 1)
                eng.drain()
                eng.wait_ge(S.qc, t + 1)
                eng.tensor_scalar_add(q1.ap(), sq.ap(), 1.0)
                eng.tensor_scalar_add(q2.ap(), rr.ap(), 1e-8)
                eng.drain()
                eng.tensor_mul(q3.ap(), q1.ap(), q2.ap())
                eng.drain()
                eng.reciprocal(qr.ap(), q3.ap())
                eng.drain()
                eng.tensor_mul(ff.ap(), sq.ap(), qr.ap())
                eng.drain()
                fb = ap3(ff, 0, [(0, C), (1, O)])
                if t < 2:
                    eng.tensor_mul(v16.ap(), ps.ap(), fb) \
                       .then_inc(S.v16sem, 1)
                    eng.maybe_drain_then_inc((S.sqdone, 1))
                else:
                    eng.tensor_mul(vf.ap(), ps.ap(), fb).then_inc(S.vfsem, 1)

            squash_dve(0)
            for t in range(2):
                for k in range(CH):
                    n = t * CH + k
                    ub = k * CG * 1024
                    if t == 0:
                        eng.wait_ge(S.evsem, CG * (k + 1))
                    if k == 0:
                        eng.wait_ge(S.vrsem, 16 * ISUB * (t + 1))
                    # agr mult: tmpa = u_chunk * vrep (bcast over g)
                    eng.tensor_mul(
                        tmpa.ap(),
                        ap3(u, ub, [(1024, CG), (1, 1024)]),
                        ap3(vrep, 0, [(0, CG), (1, 1024)]))
                    eng.drain()
                    # tree reduce over c (in-place strided)
                    eng.tensor_add(
                        ap3(tmpa, 0, [(1024, CG), (O, 8), (1, O)]),
                        ap3(tmpa, 0, [(1024, CG), (O, 8), (1, O)]),
                        ap3(tmpa, 512, [(1024, CG), (O, 8), (1, O)]))
                    eng.drain()
                    eng.tensor_add(
                        ap3(tmpa, 0, [(1024, CG), (O, 4), (1, O)]),
                        ap3(tmpa, 0, [(1024, CG), (O, 4), (1, O)]),
                        ap3(tmpa, 256, [(1024, CG), (O, 4), (1, O)]))
                    eng.drain()
                    eng.tensor_add(
                        ap3(tmpa, 0, [(1024, CG), (O, 2), (1, O)]),
                        ap3(tmpa, 0, [(1024, CG), (O, 2), (1, O)]),
                        ap3(tmpa, 128, [(1024, CG), (O, 2), (1, O)]))
                    eng.drain()
                    lsl = ap3(L, k * CG * O, [(O, CG), (1, O)])
                    t3a = ap3(tmpa, 0, [(1024, CG), (1, O)])
                    t3b = ap3(tmpa, O, [(1024, CG), (1, O)])
                    if t == 0:
                        eng.tensor_add(lsl, t3a, t3b).then_inc(S.Lsem, 1)
                    else:
                        eng.tensor_add(Ltmp.ap(), t3a, t3b)
                        eng.drain()
                        eng.tensor_add(lsl, lsl, Ltmp.ap()) \
                           .then_inc(S.Lsem, 1)
                    # softmax pieces
                    eng.wait_ge(S.xsem, n + 1)
                    eng.reduce_sum(
                        ap3(Zb, k * CG, [(1, CG)]),
                        ap3(eb[n % 2], 0, [(O, CG), (1, O)]),
                        axis=AX.X)
                    eng.drain()
                    eng.reciprocal(ap3(zr, k * CG, [(1, CG)]),
                                   ap3(Zb, k * CG, [(1, CG)]))
                    eng.drain()
                    eng.tensor_mul(
                        cbuf.ap(),
                        ap3(eb[n % 2], 0, [(O, CG), (1, O)]),
                        ap3(zr, k * CG, [(1, CG), (0, O)])) \
                        .then_inc(S.ebfree, 1)
                    eng.drain()
                    # s mult: tmps = u_chunk * c (bcast over c dim)
                    if n >= 2:
                        eng.wait_ge(S.tmpsfree, n - 1)
                    eng.tensor_mul(
                        tmps[k % 2].ap(),
                        ap3(u, ub, [(1024, CG), (O, C), (1, O)]),
                        ap3(cbuf, 0, [(O, CG), (0, C), (1, O)])) \
                        .then_inc(S.smsem, 1)
                squash_dve(t + 1)

    return nc


def _preprocess(x, W):
    """Host-side repack (fp16 casts + layout) -> per-core input maps."""
    f16 = np.float16
    # W tiles: [g, (i_sub*8+d), (c*64+o)]
    Wt = np.ascontiguousarray(
        W.reshape(G, ISUB, O, D, C).transpose(0, 1, 3, 4, 2)
        .reshape(G, 128, 1024)).astype(f16)
    in_maps = []
    sel = np.zeros((128, NB), f16)
    sel[np.arange(128), np.arange(128) % NB] = 1.0
    for core in range(NCORES):
        xc = x[core * NB:(core + 1) * NB]            # [8, 1152, 8]
        xr = xc.reshape(NB, G, ISUB, D)              # (b, g, i_sub, d)
        xbd = np.zeros((G, 128, 128), f16)
        for isub in range(ISUB):
            xbd[:, isub * D:(isub + 1) * D, isub * NB:(isub + 1) * NB] = \
                xr[:, :, isub, :].transpose(1, 2, 0)  # (g, d, b)
        WXc = np.concatenate([Wt, xbd], axis=2)      # [72, 128, 1152]
        xdn = np.ascontiguousarray(
            (xr / 64.0).transpose(2, 3, 1, 0).reshape(128, G * NB)).astype(f16)
        in_maps.append({'WX': WXc, 'XDN': xdn, 'SEL': sel})
    return in_maps


def _postprocess(results):
    out = np.empty((NCORES * NB, O, C), np.float32)
    for core in range(NCORES):
        vo = results[core]['VOUT']                   # [8, 1024] = (c, o)
        out[core * NB:(core + 1) * NB] = \
            vo.reshape(NB, C, O).transpose(0, 2, 1)
    return out


def kernel(x, W):
    from concourse.bass_utils import run_bass_kernel_spmd
    x = np.asarray(x, np.float32)
    W = np.asarray(W, np.float32)
    if 'nc' not in _cache:
        _cache['nc'] = _build_program()
    in_maps = _preprocess(x, W)
    res = run_bass_kernel_spmd(_cache['nc'], in_maps,
                               core_ids=list(range(NCORES)))
    return _postprocess(res.results)


def kernel_sim(x, W, core=0):
    """CoreSim single-core check: returns v for that core's 8 batch rows."""
    from concourse import bass_interp
    x = np.asarray(x, np.float32)
    W = np.asarray(W, np.float32)
    if 'nc' not in _cache:
        _cache['nc'] = _build_program()
    in_maps = _preprocess(x, W)
    sim = bass_interp.CoreSim(_cache['nc'])
    for name, arr in in_maps[core].items():
        sim.tensor(name)[:] = arr
    sim.simulate()
    vo = np.asarray(sim.tensor('VOUT'))
    return vo.reshape(NB, C, O).transpose(0, 2, 1)


# revision 19
# speedup vs baseline: 10565.8923x; 10565.8923x over previous
"""CapsuleLayer (dynamic routing) Trainium2 kernel.

Problem: x [64,1152,8] f32, W [1152,64,8,16] f32 ->
  u_hat = einsum('bid,iodc->bioc', x, W)
  3 routing iterations (softmax over o=64, weighted i-sum, squash, agreement)
  returns v [64,64,16] f32.

Sharding: data-parallel over batch, 8 batch elements per core x 8 cores.

Per-core device strategy (raw bass, static program, manual semaphores):
  Phase 1: stream 72 fused tiles WX[g] = [W_tile | xbd_tile] fp16 [128,1152]
    (3-slot pipeline). PE: u_hat psum tiles (block-diag x) + s0 accumulation
    (uniform c0 folded into xdn/64). Evac psum->SBUF fp16 split ACT/DVE.
  Routing iters t=0,1 over 18 chunks of 4 groups; chunks are split between
    DVE (10) and GpSimd (8): owner does agr-mult + c-tree + logit update +
    s-mult; DVE does all softmax pieces; PE reduces partitions (selector)
    accumulating s in psum; ACT does exp. DVE pipeline is reordered
    (softmax of chunk k-1 after agr of chunk k) to hide ACT latency.
  Squash on ACT/DVE; v replicated to 128 partitions via 16 small DMAs.

Precision (validated vs f32 reference in numpy: rel err ~5e-4):
  fp16 inputs/u_hat/logits/exp/c, f32 psum accumulation and squash math.
"""

import numpy as np

NB = 8        # batch per core
NCORES = 8
G = 72        # i-groups of 16 in-capsules
CG = 4        # groups per routing chunk
CH = G // CG  # 18 chunks
O, C, D = 64, 16, 8
ISUB = 16     # in-caps per group
NWX = 4       # WX pipeline slots

POOL_CHUNKS = (1, 2, 3, 5, 6, 7, 9, 10, 11, 13, 14, 15)
PP = len(POOL_CHUNKS)          # 8 pool chunks per iter
DD = CH - PP                   # 10 dve chunks per iter


def _is_pool(k):
    return k in POOL_CHUNKS


def _cnt_p(k):
    return sum(1 for j in POOL_CHUNKS if j <= k)


def _cnt_d(k):
    return (k + 1) - _cnt_p(k)


_cache = {}
PARANOID = [False]  # True: emit same-engine drains for CoreSim race detector



def _build_program(paranoid=False):
    import concourse.bass as bass
    import concourse.mybir as mybir

    f16 = mybir.dt.float16
    f32 = mybir.dt.float32

    nc = bass.Bass('TRN2', target_bir_lowering=False, debug=False)

    # ---- DRAM I/O ----
    WX = nc.dram_tensor('WX', [G, 128, 1152], f16, kind='ExternalInput')
    XDN = nc.dram_tensor('XDN', [128, G * NB], f16, kind='ExternalInput')
    SEL = nc.dram_tensor('SEL', [128, NB], f16, kind='ExternalInput')
    VOUT = nc.dram_tensor('VOUT', [NB, 1024], f32, kind='ExternalOutput')

    # ---- SBUF ----
    u = nc.alloc_sbuf_tensor('u', [128, G * 1024], f16)          # 144KB/part
    wxs = [nc.alloc_sbuf_tensor('wx%d' % i, [128, 1152], f16)
           for i in range(NWX)]
    xdn = nc.alloc_sbuf_tensor('xdn', [128, G * NB], f16)
    sel = nc.alloc_sbuf_tensor('sel', [128, NB], f16)
    L = nc.alloc_sbuf_tensor('L', [128, G * O], f16)             # 9KB
    Ltmp = nc.alloc_sbuf_tensor('Ltmp', [128, CG * O], f16)
    cb = [nc.alloc_sbuf_tensor('cb%d' % i, [128, CG * O], f16)
          for i in range(2)]
    tmpa = nc.alloc_sbuf_tensor('tmpa', [128, CG * 1024], f16)   # 8KB
    tmps = [nc.alloc_sbuf_tensor('tmps%d' % i, [128, CG * 1024], f16)
            for i in range(2)]
    eb = [nc.alloc_sbuf_tensor('eb%d' % i, [128, CG * O], f16)
          for i in range(2)]
    Zb = nc.alloc_sbuf_tensor('Zb', [128, G], f32)
    zr = nc.alloc_sbuf_tensor('zr', [128, G], f32)
    vrep = nc.alloc_sbuf_tensor('vrep', [128, 1024], f16)
    v16 = nc.alloc_sbuf_tensor('v16', [NB, 1024], f16)
    s2 = nc.alloc_sbuf_tensor('s2', [NB, 1024], f32)             # also vf
    sq = nc.alloc_sbuf_tensor('sq', [NB, O], f32)
    rr = nc.alloc_sbuf_tensor('rr', [NB, O], f32)
    q1 = nc.alloc_sbuf_tensor('q1', [NB, O], f32)
    q2 = nc.alloc_sbuf_tensor('q2', [NB, O], f32)
    ff = nc.alloc_sbuf_tensor('ff', [NB, O], f32)
    vf = s2  # s2's last read (the sq reduce) precedes the vf write

    # ---- PSUM ----
    pg0 = nc.alloc_psum_tensor('pg0', [128, 1024], f32)
    pg1 = nc.alloc_psum_tensor('pg1', [128, 1024], f32)
    ps = nc.alloc_psum_tensor('ps', [NB, 1024], f32)
    pg = [pg0, pg1]

    AF = mybir.ActivationFunctionType
    AX = mybir.AxisListType

    sems = {}
    for name in ['d0', 'wxfree', 'pgsem', 'evsemA', 'evsemD',
                 'ssem', 'qa', 'qb', 'qc', 'sqdone', 'v16sem', 'vfsem',
                 'vrsem', 'Lsem_d', 'Lsem_p', 'xsem', 'ebfree', 'csem',
                 'smsem_d', 'smsem_p', 'tmpsfree', 'dout']:
        sems[name] = nc.alloc_semaphore(name)
    wxsems = [nc.alloc_semaphore('wxs%d' % i) for i in range(NWX)]
    S = type('S', (), sems)

    def ap3(t, base, dims):
        # strided view: dims = [(step, count), ...] on free axis
        a = t.ap()
        return bass.AP(a.tensor, base, [a.ap[0]] + [[s, n] for s, n in dims])

    def dr(eng):
        # same-engine RAW ordering is guaranteed by in-order engines with
        # per-op pipeline drain; explicit drains only appease the race
        # detector in CoreSim builds.
        if paranoid:
            eng.drain()

    def agr_block(eng, t, k, buf, ltbuf, lsem):
        """agreement mult + c-tree + logit update for chunk k, iter t."""
        ub = k * CG * 1024
        eng.tensor_mul(
            buf.ap(),
            ap3(u, ub, [(1024, CG), (1, 1024)]),
            ap3(vrep, 0, [(0, CG), (1, 1024)]))
        dr(eng)
        eng.tensor_add(
            ap3(buf, 0, [(1024, CG), (O, 8), (1, O)]),
            ap3(buf, 0, [(1024, CG), (O, 8), (1, O)]),
            ap3(buf, 512, [(1024, CG), (O, 8), (1, O)]))
        dr(eng)
        eng.tensor_add(
            ap3(buf, 0, [(1024, CG), (O, 4), (1, O)]),
            ap3(buf, 0, [(1024, CG), (O, 4), (1, O)]),
            ap3(buf, 256, [(1024, CG), (O, 4), (1, O)]))
        dr(eng)
        eng.tensor_add(
            ap3(buf, 0, [(1024, CG), (O, 2), (1, O)]),
            ap3(buf, 0, [(1024, CG), (O, 2), (1, O)]),
            ap3(buf, 128, [(1024, CG), (O, 2), (1, O)]))
        dr(eng)
        lsl = ap3(L, k * CG * O, [(O, CG), (1, O)])
        t3a = ap3(buf, 0, [(1024, CG), (1, O)])
        t3b = ap3(buf, O, [(1024, CG), (1, O)])
        if t == 0:
            eng.tensor_add(lsl, t3a, t3b).then_inc(lsem, 1)
        else:
            eng.tensor_add(ltbuf.ap(), t3a, t3b)
            dr(eng)
            eng.tensor_add(lsl, lsl, ltbuf.ap()).then_inc(lsem, 1)
        dr(eng)

    def smult(eng, t, k, smsem):
        """s-mult for chunk k: tmps[n%2] = u_chunk * c (bcast over c)."""
        n = t * CH + k
        if n >= 2:
            eng.wait_ge(S.tmpsfree, n - 1)
        eng.tensor_mul(
            tmps[n % 2].ap(),
            ap3(u, k * CG * 1024, [(1024, CG), (O, C), (1, O)]),
            ap3(cb[n % 2], 0, [(O, CG), (0, C), (1, O)])) \
            .then_inc(smsem, 1)
        dr(eng)

    with nc.allow_low_precision(reason='fp16 validated to 5e-4 vs f32 ref'), \
         nc.Block() as block:

        # ---------------- SYNC: all DMA ----------------
        @block.sync
        def _(eng):
            eng.dma_start(sel.ap(), SEL.ap()).then_inc(S.d0, 16)
            eng.dma_start(xdn.ap(), XDN.ap()).then_inc(S.d0, 16)
            for g in range(G):
                if g >= NWX:
                    eng.wait_ge(S.wxfree, g - NWX + 1)
                eng.dma_start(wxs[g % NWX].ap(), WX.ap()[g]) \
                   .then_inc(wxsems[g % NWX], 16)
            for t in range(2):
                eng.wait_ge(S.v16sem, t + 1)
                if t == 1:
                    eng.wait_ge(S.Lsem_d, CH)
                for isub in range(ISUB):
                    eng.dma_start(vrep.ap()[isub * NB:(isub + 1) * NB, :],
                                  v16.ap()).then_inc(S.vrsem, 16)
            eng.wait_ge(S.vfsem, 1)
            eng.dma_start(VOUT.ap(), vf.ap()).then_inc(S.dout, 16)

        # ---------------- PE ----------------
        @block.tensor
        def _(eng):
            for g in range(G):
                b = g % NWX
                eng.wait_ge(wxsems[b], 16 * (g // NWX + 1))
                if g >= 2:
                    gp = g - 2  # evac owner of pg slot being overwritten
                    if gp % 2 == 0:
                        eng.wait_ge(S.evsemA, gp // 2 + 1)
                    else:
                        eng.wait_ge(S.evsemD, (gp + 1) // 2)
                eng.matmul(pg[g % 2].ap()[:, 0:512],
                           lhsT=wxs[b].ap()[:, 1024:1152],
                           rhs=wxs[b].ap()[:, 0:512], start=True, stop=True)
                eng.matmul(pg[g % 2].ap()[:, 512:1024],
                           lhsT=wxs[b].ap()[:, 1024:1152],
                           rhs=wxs[b].ap()[:, 512:1024],
                           start=True, stop=True).then_inc(S.pgsem, 1)
                if g == 0:
                    eng.wait_ge(S.d0, 32)
                eng.matmul(ps.ap()[:, 0:512],
                           lhsT=xdn.ap()[:, g * NB:(g + 1) * NB],
                           rhs=wxs[b].ap()[:, 0:512],
                           start=(g == 0), stop=(g == G - 1))
                eng.matmul(ps.ap()[:, 512:1024],
                           lhsT=xdn.ap()[:, g * NB:(g + 1) * NB],
                           rhs=wxs[b].ap()[:, 512:1024],
                           start=(g == 0), stop=(g == G - 1)) \
                   .then_inc(S.wxfree, 1)
                if g == G - 1:
                    eng.maybe_drain_then_inc((S.ssem, 1))
            for t in range(2):
                eng.wait_ge(S.sqdone, t + 1)
                for k in range(CH):
                    n = t * CH + k
                    if _is_pool(k):
                        eng.wait_ge(S.smsem_p, PP * t + _cnt_p(k))
                    else:
                        eng.wait_ge(S.smsem_d, DD * t + _cnt_d(k))
                    for gs in range(CG):
                        for h in range(2):
                            mm = eng.matmul(
                                ps.ap()[:, h * 512:(h + 1) * 512],
                                lhsT=sel.ap(),
                                rhs=tmps[n % 2].ap()[:, gs * 1024 + h * 512:
                                                     gs * 1024 + (h + 1) * 512],
                                start=(k == 0 and gs == 0),
                                stop=(k == CH - 1 and gs == CG - 1))
                    mm.then_inc(S.tmpsfree, 1)
                    if k == CH - 1:
                        eng.maybe_drain_then_inc((S.ssem, 1))

        # ---------------- ACT (scalar) ----------------
        @block.scalar
        def _(eng):
            def squash_act(t):
                eng.wait_ge(S.ssem, t + 1)
                eng.activation(s2.ap(), ps.ap(), AF.Square).then_inc(S.qa, 1)
                eng.wait_ge(S.qb, t + 1)
                eng.activation(rr.ap(), sq.ap(), AF.Sqrt).then_inc(S.qc, 1)

            for g in range(0, G, 2):   # even g evac
                eng.wait_ge(S.pgsem, g + 1)
                eng.activation(ap3(u, g * 1024, [(1, 1024)]),
                               pg[g % 2].ap(), AF.Copy).then_inc(S.evsemA, 1)
            squash_act(0)
            for t in range(2):
                for k in range(CH):
                    n = t * CH + k
                    eng.wait_ge(S.Lsem_d, n + 1)
                    if n >= 2:
                        eng.wait_ge(S.ebfree, n - 1)
                    for gi in range(CG):
                        a = eng.activation(
                            ap3(eb[n % 2], gi * O, [(1, O)]),
                            ap3(L, k * CG * O + gi * O, [(1, O)]),
                            AF.Exp,
                            accum_out=ap3(Zb, k * CG + gi, [(1, 1)]))
                    a.then_inc(S.xsem, 1)
                squash_act(t + 1)

        # ---------------- DVE (vector) ----------------
        @block.vector
        def _(eng):
            def squash_dve(t):
                eng.wait_ge(S.qa, t + 1)
                eng.reduce_sum(sq.ap(),
                               ap3(s2, 0, [(1, O), (O, C)]),
                               axis=AX.X).then_inc(S.qb, 1)
                dr(eng)
                eng.wait_ge(S.qc, t + 1)
                eng.tensor_scalar_add(q1.ap(), sq.ap(), 1.0)
                eng.tensor_scalar_add(q2.ap(), rr.ap(), 1e-8)
                dr(eng)
                eng.tensor_mul(q1.ap(), q1.ap(), q2.ap())
                dr(eng)
                eng.reciprocal(q2.ap(), q1.ap())
                dr(eng)
                eng.tensor_mul(ff.ap(), sq.ap(), q2.ap())
                dr(eng)
                fb = ap3(ff, 0, [(0, C), (1, O)])
                if t < 2:
                    eng.tensor_mul(v16.ap(), ps.ap(), fb) \
                       .then_inc(S.v16sem, 1)
                    eng.maybe_drain_then_inc((S.sqdone, 1))
                else:
                    eng.tensor_mul(vf.ap(), ps.ap(), fb).then_inc(S.vfsem, 1)

            def softmax_smult(t, j):
                nj = t * CH + j
                eng.wait_ge(S.xsem, nj + 1)
                eng.reciprocal(ap3(zr, j * CG, [(1, CG)]),
                               ap3(Zb, j * CG, [(1, CG)]))
                dr(eng)
                j2 = j - 2  # cbuf slot WAR vs pool reader two chunks back
                if j2 >= 0 and _is_pool(j2):
                    eng.wait_ge(S.smsem_p, PP * t + _cnt_p(j2))
                eng.tensor_mul(cb[nj % 2].ap(),
                               ap3(eb[nj % 2], 0, [(O, CG), (1, O)]),
                               ap3(zr, j * CG, [(1, CG), (0, O)]))
                eng.sem_inc(S.ebfree, 1)
                eng.maybe_drain_then_inc((S.csem, 1))
                if not _is_pool(j):
                    smult(eng, t, j, S.smsem_d)

            for g in range(1, G, 2):   # odd g evac
                eng.wait_ge(S.pgsem, g + 1)
                eng.tensor_copy(ap3(u, g * 1024, [(1, 1024)]),
                                pg[g % 2].ap()).then_inc(S.evsemD, 1)
            squash_dve(0)
            for t in range(2):
                for k in range(CH):
                    if t == 0:
                        eng.wait_ge(S.evsemA, 2 * k + 2)
                        eng.wait_ge(S.evsemD, 2 * k + 2)
                    if k == 0:
                        eng.wait_ge(S.vrsem, 16 * ISUB * (t + 1))
                    agr_block(eng, t, k, tmpa, Ltmp, S.Lsem_d)
                    if k > 0:
                        softmax_smult(t, k - 1)
                softmax_smult(t, CH - 1)
                squash_dve(t + 1)

        # ---------------- GpSimd (pool): s-mults only ----------------
        @block.gpsimd
        def _(eng):
            for t in range(2):
                for k in POOL_CHUNKS:
                    n = t * CH + k
                    if t == 0:
                        eng.wait_ge(S.evsemA, 2 * k + 2)
                        eng.wait_ge(S.evsemD, 2 * k + 2)
                    eng.wait_ge(S.csem, n + 1)
                    smult(eng, t, k, S.smsem_p)

    return nc


def _preprocess(x, W):
    """Host-side repack (fp16 casts + layout) -> per-core input maps."""
    f16 = np.float16
    # W tiles: [g, (i_sub*8+d), (c*64+o)]
    Wt = np.ascontiguousarray(
        W.reshape(G, ISUB, O, D, C).transpose(0, 1, 3, 4, 2)
        .reshape(G, 128, 1024)).astype(f16)
    in_maps = []
    sel = np.zeros((128, NB), f16)
    sel[np.arange(128), np.arange(128) % NB] = 1.0
    for core in range(NCORES):
        xc = x[core * NB:(core + 1) * NB]            # [8, 1152, 8]
        xr = xc.reshape(NB, G, ISUB, D)              # (b, g, i_sub, d)
        xbd = np.zeros((G, 128, 128), f16)
        for isub in range(ISUB):
            xbd[:, isub * D:(isub + 1) * D, isub * NB:(isub + 1) * NB] = \
                xr[:, :, isub, :].transpose(1, 2, 0)  # (g, d, b)
        WXc = np.concatenate([Wt, xbd], axis=2)      # [72, 128, 1152]
        xdn = np.ascontiguousarray(
            (xr / 64.0).transpose(2, 3, 1, 0).reshape(128, G * NB)).astype(f16)
        in_maps.append({'WX': WXc, 'XDN': xdn, 'SEL': sel})
    return in_maps


def _postprocess(results):
    out = np.empty((NCORES * NB, O, C), np.float32)
    for core in range(NCORES):
        vo = results[core]['VOUT']                   # [8, 1024] = (c, o)
        out[core * NB:(core + 1) * NB] = \
            vo.reshape(NB, C, O).transpose(0, 2, 1)
    return out


def kernel(x, W):
    from concourse.bass_utils import run_bass_kernel_spmd
    x = np.asarray(x, np.float32)
    W = np.asarray(W, np.float32)
    if 'nc' not in _cache:
        # paranoid=True: same-engine drains are required on hardware too —
        # verified empirically (drain-stripped build returns garbage).
        _cache['nc'] = _build_program(paranoid=True)
    in_maps = _preprocess(x, W)
    res = run_bass_kernel_spmd(_cache['nc'], in_maps,
                               core_ids=list(range(NCORES)))
    return _postprocess(res.results)


def kernel_sim(x, W, core=0):
    """CoreSim single-core check: returns v for that core's 8 batch rows."""
    from concourse import bass_interp
    x = np.asarray(x, np.float32)
    W = np.asarray(W, np.float32)
    if 'nc_sim' not in _cache:
        _cache['nc_sim'] = _build_program(paranoid=True)
    in_maps = _preprocess(x, W)
    sim = bass_interp.CoreSim(_cache['nc_sim'])
    for name, arr in in_maps[core].items():
        sim.tensor(name)[:] = arr
    sim.simulate()
    vo = np.asarray(sim.tensor('VOUT'))
    return vo.reshape(NB, C, O).transpose(0, 2, 1)
